# revision 6
# baseline (speedup 1.0000x reference)
"""Multi-head causal attention (B=4, T=2048, H=16, D=64) on 8 trn2 NeuronCores.

Sharding: core c = (batch b = c//2, head-group hg = c%2 of 8 heads).
Each core computes its batch's QKV projection for its 8 heads, causal
attention, and a partial output projection (contraction over its 512
channels of W_proj). Host sums the two partials per batch and adds bias.

Per-core kernel layout choices:
  - All HBM tensors are host-side re-laid-out so every DMA moves one
    contiguous 2-8KB chunk per partition (descriptor-efficient).
  - K^T, Q^T stored [hd, t] with head-dim on partitions (64 per head, 2
    heads per 128-partition tile); the per-head score matmul pair (K=64)
    sits on disjoint 64-row PE tiles and dual-issues on HW.
  - V stored [t, h*65+d] with a ones column appended per head -> the AV
    matmul O_T = V_aug^T(stationary) x P_T produces softmax denominators
    in row 64 for free.
  - Scores are computed transposed S_T[k, q]; P_T = exp(S_T) feeds AV
    directly; no max subtraction needed (|scores/8| < ~3).
  - Causal masking of diagonal blocks is an in-place affine_select on
    GpSimd (otherwise idle); softmax reciprocal reads PSUM directly.
  - O_T [hd, t] is exactly the lhsT the output projection needs.

Scheduling: the attention inner loop is ACT(exp)-bound while QKV/proj
are PE-bound.  QKV(tb+1)/proj(qb) matmuls are queued as "filler" units
and injected between attention steps via a virtual-clock deficit so the
PE never idles waiting for exps.  Chains are flushed just-in-time one
pair ahead of their consumer.  proj(2) and the ct={0,1} halves of
proj(3) fill the ACT-bound qb=3 pairs; the tail then only runs the
ct={2,3} halves (8 short chains) + adds, keeping the PE warm to the end.
"""

import os
import sys

import numpy as np

F16_NP = np.dtype(np.float16)

if "/opt/trn_rl_repo" not in sys.path:
    sys.path.insert(0, "/opt/trn_rl_repo")

from collections import deque
from contextlib import ExitStack

import concourse.bass as bass
import concourse.bacc as bacc
import concourse.mybir as mybir
import concourse.tile as tile
from concourse._compat import with_exitstack

P = 128
T = 2048
C = 1024
H_PER_CORE = 8
D = 64
DP = D + 1  # V augmented with a ones column
NC_CORES = 8

TB = 4  # t-blocks of 512
QB = 4  # q-blocks of 512
CI = 8  # contraction tiles of 128 over C for QKV proj

F32 = mybir.dt.float32
F16 = mybir.dt.float16

# virtual-clock cost constants (ns) for filler pacing
PE_CYC = 0.4167
ACT_COL = 0.8333
ACT_OVH = 350.0
STEP_PE_OVH = 35.0
MM_NS = 512 * PE_CYC + 20.0


@with_exitstack
def build_attention_kernel(ctx: ExitStack, tc: tile.TileContext):
    nc = tc.nc

    # HBM layouts (host pre-permuted for contiguous per-partition lines):
    #   x4[ci, tb, co, t]   : x^T channel c = co*128+ci, time t = tb*512+ts
    #   wk4/wq4[mc, ci, co, m]: W[:, mc*128+m] column-chunked
    #   wv2[ci, co, m]      : full [1024, 512] V weight
    #   wp2[ci, co, n]      : [512, 1024] proj weight
    #   y4[p, tt, n]        : output, t = tt*128 + p
    x4 = nc.declare_dram_parameter("x4", [P, TB, CI, 512], F16, isOutput=False)
    wk4 = nc.declare_dram_parameter("wk4", [4, P, CI, P], F16, isOutput=False)
    wq4 = nc.declare_dram_parameter("wq4", [4, P, CI, P], F16, isOutput=False)
    wv2 = nc.declare_dram_parameter("wv2", [P, CI, 512], F16, isOutput=False)
    wp2 = nc.declare_dram_parameter("wp2", [P, 4, C], F16, isOutput=False)
    y4 = nc.declare_dram_parameter("y4", [P, 16, C], F16, isOutput=True)

    # ---- pools ----
    kt_pool = ctx.enter_context(tc.tile_pool(name="ktp", bufs=16))
    qt_pool = ctx.enter_context(tc.tile_pool(name="qtp", bufs=16))
    ot_pool = ctx.enter_context(tc.tile_pool(name="otp", bufs=16))
    v_pool = ctx.enter_context(tc.tile_pool(name="vp", bufs=4))
    w_pool = ctx.enter_context(tc.tile_pool(name="wp_", bufs=1))
    xt_pool = ctx.enter_context(tc.tile_pool(name="xtp", bufs=4))
    pt_pool = ctx.enter_context(tc.tile_pool(name="ptp", bufs=8))
    recip_pool = ctx.enter_context(tc.tile_pool(name="recipp", bufs=6))
    bc_pool = ctx.enter_context(tc.tile_pool(name="bcp", bufs=4))
    y_pool = ctx.enter_context(tc.tile_pool(name="yp", bufs=4))
    yh_pool = ctx.enter_context(tc.tile_pool(name="yhp", bufs=8))
    # PSUM: 8 banks total: s-pairs 2x2, AV accum 2x1, qkv/proj 2x1
    ps_s_pool = ctx.enter_context(tc.tile_pool(name="ps_s", bufs=2, space="PSUM"))
    ps_o_pool = ctx.enter_context(tc.tile_pool(name="ps_o", bufs=2, space="PSUM"))
    ps_q_pool = ctx.enter_context(tc.tile_pool(name="ps_q", bufs=2, space="PSUM"))

    # KT[pt][tb], QT[pt][qb]: [128, 512]; partitions = 2 heads x 64 dims
    KT = [[kt_pool.tile([P, 512], F16, tag="kt", name=f"KT_{pt}_{tb}") for tb in range(TB)] for pt in range(4)]
    QT = [[qt_pool.tile([P, 512], F16, tag="qt", name=f"QT_{pt}_{qb}") for qb in range(QB)] for pt in range(4)]
    OT = [[ot_pool.tile([P, 512], F16, tag="ot", name=f"OT_{hp}_{qb}") for qb in range(QB)] for hp in range(4)]
    V = [v_pool.tile([P, 4, H_PER_CORE * DP], F16, tag="v", name=f"V_{tb}") for tb in range(TB)]
    wk_sb = w_pool.tile([P, 4, CI, P], F16)
    wq_sb = w_pool.tile([P, 4, CI, P], F16)
    wv_sb = w_pool.tile([P, CI, 512], F16)
    wp_sb = w_pool.tile([P, 4, C], F16)

    # ---- prologue DMAs: three HWDGE queues, need-ordered; every transfer
    # is per-partition contiguous so descriptors are 2-8KB each ----
    xts = {}
    for tb in range(TB):
        xts[tb] = xt_pool.tile([P, CI, 512], F16, tag="xt", name=f"xt{tb}")
    # sync + scalar (HWDGE) queues carry the critical path, need-ordered;
    # gpsimd (SWDGE) carries wv/wp which have several-us of slack
    nc.sync.dma_start(wk_sb[:, 0], wk4[0])
    nc.scalar.dma_start(xts[0][:, 0:2], x4[:, 0, 0:2])
    nc.sync.dma_start(xts[0][:, 4:6], x4[:, 0, 4:6])
    nc.scalar.dma_start(xts[0][:, 2:4], x4[:, 0, 2:4])
    nc.sync.dma_start(xts[0][:, 6:8], x4[:, 0, 6:8])
    nc.scalar.dma_start(wq_sb[:, 0], wq4[0])
    nc.gpsimd.dma_start(wv_sb[:, :, 0:256], wv2[:, :, 0:256])
    nc.gpsimd.dma_start(wv_sb[:, :, 256:512], wv2[:, :, 256:512])
    nc.sync.dma_start(wk_sb[:, 1], wk4[1])
    nc.scalar.dma_start(wq_sb[:, 1], wq4[1])
    nc.sync.dma_start(xts[1][:, 4:8], x4[:, 1, 4:8])
    nc.scalar.dma_start(xts[1][:, 0:4], x4[:, 1, 0:4])
    nc.sync.dma_start(wk_sb[:, 2:4], wk4[2:4].rearrange("mc ci co m -> ci mc co m"))
    nc.scalar.dma_start(wq_sb[:, 2:4], wq4[2:4].rearrange("mc ci co m -> ci mc co m"))
    nc.gpsimd.dma_start(wp_sb[:], wp2[:])
    nc.sync.dma_start(xts[2][:, 4:8], x4[:, 2, 4:8])
    nc.scalar.dma_start(xts[2][:, 0:4], x4[:, 2, 0:4])
    nc.sync.dma_start(xts[3][:, 4:8], x4[:, 3, 4:8])
    nc.scalar.dma_start(xts[3][:, 0:4], x4[:, 3, 0:4])

    # diagonal causal mask patterns are applied in-place on GpSimd
    # (affine_select); only the ones column of V needs a memset.
    for tb in range(TB):
        ones_col = V[tb].rearrange("p s (h e) -> p s h e", e=DP)[:, :, :, D : D + 1]
        nc.gpsimd.memset(ones_col, 1.0)

    # ================= filler machinery =================
    filler_q = deque()  # entries: (tag, sub, pe_ns, closure)
    clk = {"deficit": 0.0}

    def emit_fillers(gap_ns):
        clk["deficit"] += gap_ns
        while clk["deficit"] > 0.0 and filler_q:
            _, _, pe_ns, fn = filler_q.popleft()
            fn()
            clk["deficit"] -= pe_ns

    def flush_sub(tag, subs):
        """Emit from the front until no unit with (tag, sub in subs) remains."""
        while any(e[0] == tag and e[1] in subs for e in filler_q):
            _, _, _, fn = filler_q.popleft()
            fn()

    def drain_all():
        while filler_q:
            _, _, _, fn = filler_q.popleft()
            fn()

    # ================= QKV chains =================
    def qkv_chain_units(tb, kind, idx, sub):
        st = {}
        tag = f"qkv{tb}"
        units = []

        def mk(ci):
            def f():
                xt = xts[tb]
                if ci == 0:
                    st["ps"] = ps_q_pool.tile([P, 512], F32, tag="psq", name=f"ps_{tag}")
                ps = st["ps"]
                if kind == "K":
                    nc.tensor.matmul(
                        ps[:], lhsT=wk_sb[:, idx, ci],
                        rhs=xt[:, ci, :], start=(ci == 0), stop=(ci == CI - 1),
                    )
                elif kind == "Q":
                    nc.tensor.matmul(
                        ps[:], lhsT=wq_sb[:, idx, ci],
                        rhs=xt[:, ci, :], start=(ci == 0), stop=(ci == CI - 1),
                    )
                else:  # V
                    nc.tensor.matmul(
                        ps[:], lhsT=xt[:, ci, idx * P : (idx + 1) * P],
                        rhs=wv_sb[:, ci, :], start=(ci == 0), stop=(ci == CI - 1),
                    )
                if ci == CI - 1:
                    if kind == "K":
                        nc.vector.tensor_copy(KT[idx][tb][:], ps[:])
                    elif kind == "Q":
                        nc.vector.tensor_copy(QT[idx][tb][:], ps[:])
                    else:
                        nc.vector.tensor_copy(
                            V[tb][:, idx].rearrange("p (h e) -> p h e", e=DP)[:, :, :D],
                            ps.rearrange("p (h d) -> p h d", d=D),
                        )
            return f

        for ci in range(CI):
            units.append((tag, sub, MM_NS, mk(ci)))
        return units

    def v_half_chain_units(tb, idx, h0):
        """V chain for heads [2*h0, 2*h0+4): N=256 half-width (startup only)."""
        st = {}

        def mk(ci):
            def f():
                xt = xts[tb]
                if ci == 0:
                    st["ps"] = ps_q_pool.tile([P, 512], F32, tag="psq", name="ps_vh")
                ps = st["ps"]
                nc.tensor.matmul(
                    ps[:, :256], lhsT=xt[:, ci, idx * P : (idx + 1) * P],
                    rhs=wv_sb[:, ci, h0 * 256 : (h0 + 1) * 256],
                    start=(ci == 0), stop=(ci == CI - 1),
                )
                if ci == CI - 1:
                    nc.vector.tensor_copy(
                        V[tb][:, idx].rearrange("p (h e) -> p h e", e=DP)[
                            :, 4 * h0 : 4 * h0 + 4, :D
                        ],
                        ps[:, :256].rearrange("p (h d) -> p h d", d=D),
                    )
            return f

        return [(f"qkv{tb}", "v", 256 * PE_CYC + 20.0, mk(ci)) for ci in range(CI)]

    def qkv_units(tb):
        """V first (long-lead deps), then per-pair (Q,K) chains."""
        units = []
        for ts in range(4):
            units += qkv_chain_units(tb, "V", ts, "v")
        for pt in range(4):
            units += qkv_chain_units(tb, "Q", pt, f"p{pt}")
            units += qkv_chain_units(tb, "K", pt, f"p{pt}")
        return units

    # ================= attention =================
    def attention_pair(qb, hp, mid_emit=None):
        # JIT prefetch: flush this pair's chains (safety) plus the next
        # pair's, so the PSUM->SBUF casts land one pair ahead of use.
        subs = {f"p{hp}"}
        if hp == 0:
            subs |= {"v", "p1"}
        elif hp < 3:
            subs.add(f"p{hp + 1}")
        flush_sub(f"qkv{qb}", subs)

        ot_ps = [
            ps_o_pool.tile([DP, 512], F32, tag="ot_ps", name=f"ot_ps_{qb}_{hp}_{i}")
            for i in range(2)
        ]
        nkt = 4 * (qb + 1)
        pts = {}

        def emit_scores_exp(kt):
            tb = kt // 4
            qs = (kt - 4 * qb) * P if kt >= 4 * qb else 0
            nq = 512 - qs
            s_ps = ps_s_pool.tile([P, 2, 512], F32, tag="s_ps", name="s_ps")
            for h2 in range(2):
                nc.tensor.matmul(
                    s_ps[:, h2, qs:],
                    lhsT=KT[hp][tb][
                        h2 * D : (h2 + 1) * D,
                        (kt % 4) * P : (kt % 4 + 1) * P,
                    ],
                    rhs=QT[hp][qb][h2 * D : (h2 + 1) * D, qs:],
                    start=True,
                    stop=True,
                )
            p_t = pt_pool.tile([P, 2, 512], F16, tag="pt", name="p_t")
            nc.scalar.activation(
                p_t[:, :, qs:],
                s_ps[:, :, qs:],
                mybir.ActivationFunctionType.Exp,
                scale=0.125,
            )
            if kt >= 4 * qb:  # diagonal: zero q < k entries on GpSimd
                j = kt - 4 * qb
                nc.gpsimd.affine_select(
                    out=p_t[:, :, qs:],
                    in_=p_t[:, :, qs:],
                    compare_op=mybir.AluOpType.is_ge,
                    fill=0.0,
                    base=qs - j * P,
                    pattern=[[0, 2], [1, nq]],
                    channel_multiplier=-1,
                )
            pts[kt] = (p_t, qs, nq)
            return nq

        def emit_av(kt):
            tb = kt // 4
            p_t, qs, nq = pts.pop(kt)
            for h2 in range(2):
                h = 2 * hp + h2
                nc.tensor.matmul(
                    ot_ps[h2][:, qs:],
                    lhsT=V[tb][:, kt % 4, h * DP : (h + 1) * DP],
                    rhs=p_t[:, h2, qs:],
                    start=(kt == 0),
                    stop=(kt == nkt - 1),
                )
            return nq

        if mid_emit is not None:
            # split form (startup): all scores/exps first so ACT streams
            # while V is still loading, then the AVs
            for kt in range(nkt):
                nq = emit_scores_exp(kt)
                emit_fillers(2 * nq * ACT_COL + ACT_OVH - nq * PE_CYC - STEP_PE_OVH)
            mid_emit()
            for kt in range(nkt):
                emit_av(kt)
        else:
            # software pipeline: S(kt+1) before AV(kt); fillers pace the gaps
            nq = emit_scores_exp(0)
            emit_fillers(2 * nq * ACT_COL + ACT_OVH - nq * PE_CYC - STEP_PE_OVH)
            for kt in range(1, nkt):
                nq_s = emit_scores_exp(kt)
                nq_a = emit_av(kt - 1)
                emit_fillers(
                    2 * nq_s * ACT_COL + ACT_OVH
                    - (nq_s + 2 * nq_a) * PE_CYC - STEP_PE_OVH
                )
            emit_av(nkt - 1)

        # normalize: divide rows 0..63 by the sums row (64)
        for h2 in range(2):
            recip = recip_pool.tile([1, 512], F32, tag="recip", name="recip")
            nc.vector.tensor_copy(recip[:], ot_ps[h2][D : D + 1, :])
            nc.vector.reciprocal_approx_fast(recip[:], recip[:])
            bc = bc_pool.tile([D, 512], F32, tag="bc", name="bc")
            nc.gpsimd.partition_broadcast(bc[:], recip[:])
            nc.vector.tensor_mul(
                OT[hp][qb][h2 * D : (h2 + 1) * D, :],
                ot_ps[h2][:D, :],
                bc[:],
            )

    # ================= output projection =================
    # qb<3: full 4-ct chains -> y tile -> per-tt DMA.
    # qb=3: ct{0,1} half-chains run as fillers during the ACT-bound final
    # pairs (partial saved to SBUF); ct{2,3} half-chains + add at the tail.
    ysbs = {}
    yhalf = {}

    def proj_units(qb):
        tag = f"proj{qb}"
        units = []
        proj_ps = {}

        def mk(tt, nb, ct):
            def f():
                if nb == 0 and ct == 0:
                    ysbs[tt] = y_pool.tile([P, C], F16, tag="ysb", name="ysb")
                st_key = (tt, nb)
                if ct == 0:
                    proj_ps[st_key] = ps_q_pool.tile(
                        [P, 512], F32, tag="psq", name=f"ps_{tag}"
                    )
                ps = proj_ps[st_key]
                nc.tensor.matmul(
                    ps[:],
                    lhsT=OT[ct][qb][:, (tt % 4) * P : (tt % 4 + 1) * P],
                    rhs=wp_sb[:, ct, nb * 512 : (nb + 1) * 512],
                    start=(ct == 0),
                    stop=(ct == 3),
                )
                if ct == 3:
                    proj_ps.pop(st_key)
                    nc.vector.tensor_copy(
                        ysbs[tt][:, nb * 512 : (nb + 1) * 512], ps[:]
                    )
                    if nb == 1:
                        nc.sync.dma_start(y4[:, tt], ysbs[tt][:])
            return f

        for tt in range(4 * qb, 4 * qb + 4):
            for nb in range(2):
                for ct in range(4):
                    units.append((tag, "p", MM_NS, mk(tt, nb, ct)))
        return units

    def proj3_first_half_units():
        """ct=0,1 partial chains for qb=3; result parked in SBUF."""
        units = []
        proj_ps = {}

        def mk(tt, nb, ct):
            def f():
                st_key = (tt, nb)
                if ct == 0:
                    proj_ps[st_key] = ps_q_pool.tile(
                        [P, 512], F32, tag="psq", name="ps_p3a"
                    )
                ps = proj_ps[st_key]
                nc.tensor.matmul(
                    ps[:],
                    lhsT=OT[ct][3][:, (tt % 4) * P : (tt % 4 + 1) * P],
                    rhs=wp_sb[:, ct, nb * 512 : (nb + 1) * 512],
                    start=(ct == 0),
                    stop=(ct == 1),
                )
                if ct == 1:
                    proj_ps.pop(st_key)
                    yh = yh_pool.tile([P, 512], F16, tag="yh", name="yh")
                    nc.vector.tensor_copy(yh[:], ps[:])
                    yhalf[st_key] = yh
            return f

        for tt in range(12, 16):
            for nb in range(2):
                for ct in range(2):
                    units.append(("proj3a", "p", MM_NS, mk(tt, nb, ct)))
        return units

    def emit_proj3_tail():
        """ct=2,3 half-chains + adds + stores, in two groups of 4 chains
        (2 ps_s tiles per group).  Each group's ct=2 MMs go first — their
        OT[2] dep is ready before norm(3,3) — then ct=3 MMs, adds, DMAs."""
        chains = [(tt, nb) for tt in range(12, 16) for nb in range(2)]
        for g in range(2):
            grp = chains[4 * g : 4 * g + 4]
            proj_ps = {}
            for i, (tt, nb) in enumerate(grp):
                if i % 2 == 0:
                    ps2 = ps_s_pool.tile([P, 2, 512], F32, tag="s_ps", name="ps_p3b")
                ps = ps2[:, i % 2, :]
                proj_ps[(tt, nb)] = ps
                nc.tensor.matmul(
                    ps,
                    lhsT=OT[2][3][:, (tt % 4) * P : (tt % 4 + 1) * P],
                    rhs=wp_sb[:, 2, nb * 512 : (nb + 1) * 512],
                    start=True, stop=False,
                )
            for tt, nb in grp:
                if nb == 0:
                    ysbs[tt] = y_pool.tile([P, C], F16, tag="ysb", name="ysb")
                ps = proj_ps[(tt, nb)]
                nc.tensor.matmul(
                    ps,
                    lhsT=OT[3][3][:, (tt % 4) * P : (tt % 4 + 1) * P],
                    rhs=wp_sb[:, 3, nb * 512 : (nb + 1) * 512],
                    start=False, stop=True,
                )
                nc.vector.tensor_add(
                    ysbs[tt][:, nb * 512 : (nb + 1) * 512], ps, yhalf.pop((tt, nb))[:]
                )
                if nb == 1:
                    nc.sync.dma_start(y4[:, tt], ysbs[tt][:])

    # ================= master schedule =================
    # tb=0: K0,V0,Q0 then pair(0,0) ASAP; V1-3 and later K/Q chains slot
    # between the early (cheap) pairs, one pair ahead of their consumers.
    def emit_chains(units):
        for u in units:
            u[3]()

    emit_chains(qkv_chain_units(0, "K", 0, "p0"))
    emit_chains(qkv_chain_units(0, "Q", 0, "p0"))
    for ts in range(4):
        emit_chains(v_half_chain_units(0, ts, 0))
    filler_q.extend(qkv_units(1))
    for hp in range(4):
        if hp == 1:  # second V half feeds pairs hp>=2, one pair ahead
            for ts in range(4):
                emit_chains(v_half_chain_units(0, ts, 1))
        if hp < 3:
            emit_chains(qkv_chain_units(0, "K", hp + 1, f"p{hp+1}"))
            emit_chains(qkv_chain_units(0, "Q", hp + 1, f"p{hp+1}"))
        attention_pair(0, hp)

    for qb in range(1, 4):
        if qb < 3:
            filler_q.extend(qkv_units(qb + 1))
            filler_q.extend(proj_units(qb - 1))
        else:
            # qb=3 is ACT-bound with no QKV left: fill it with proj(2)
            # and the first halves of proj(3)
            filler_q.extend(proj_units(2))
            filler_q.extend(proj3_first_half_units())
        for hp in range(4):
            attention_pair(qb, hp)
    drain_all()
    emit_proj3_tail()

    return nc


_CACHED_NC = None


def get_nc():
    global _CACHED_NC
    if _CACHED_NC is None:
        nc = bacc.Bacc()
        with tile.TileContext(nc) as tc:
            build_attention_kernel(tc)
        nc.compile()
        _CACHED_NC = nc
    return _CACHED_NC


def make_in_maps(x, W_att, W_proj):
    x = np.asarray(x, dtype=np.float32)
    W_att = np.asarray(W_att, dtype=np.float32)
    W_proj = np.asarray(W_proj, dtype=np.float32)
    in_maps = []
    for c in range(NC_CORES):
        b, hg = c // 2, c % 2
        s = hg * 512
        # x4[ci, tb, co, t]: x^T[c, t] with c = co*128+ci, t = tb*512+ts
        xT = np.ascontiguousarray(x[b].T).astype(F16_NP)  # [1024, 2048]
        x4 = np.ascontiguousarray(
            xT.reshape(CI, P, TB, 512).transpose(1, 2, 0, 3)
        )
        # w4[mc, ci, co, m]: W[c, s + mc*128 + m]
        def w4(w):  # w: [1024, 512]
            return np.ascontiguousarray(
                w.reshape(CI, P, 4, P).transpose(2, 1, 0, 3).astype(F16_NP)
            )
        wk = W_att[:, 0 * C + s : 0 * C + s + 512].astype(F16_NP)
        wq = W_att[:, 1 * C + s : 1 * C + s + 512].astype(F16_NP)
        wv = W_att[:, 2 * C + s : 2 * C + s + 512].astype(F16_NP)
        # wv2[ci, co, m]
        wv2 = np.ascontiguousarray(wv.reshape(CI, P, 512).transpose(1, 0, 2))
        # wp2[ci, co, n]: W_proj[s + co*128 + ci, n]
        wp2 = np.ascontiguousarray(
            W_proj[s : s + 512].astype(F16_NP).reshape(4, P, C).transpose(1, 0, 2)
        )
        in_maps.append(
            {"x4": x4, "wk4": w4(wk), "wq4": w4(wq), "wv2": wv2, "wp2": wp2}
        )
    return in_maps


def combine_outputs(results, b_proj):
    B = NC_CORES // 2
    out = np.empty((B, T, C), dtype=np.float32)
    bias = np.asarray(b_proj, dtype=np.float32)
    for b in range(B):
        # y4[p, tt, n] -> y[t, n] with t = tt*128 + p
        ya = results[2 * b]["y4"].astype(np.float32).transpose(1, 0, 2).reshape(T, C)
        yb = results[2 * b + 1]["y4"].astype(np.float32).transpose(1, 0, 2).reshape(T, C)
        out[b] = ya + yb + bias
    return out


def kernel(x, W_att, W_proj, b_proj):
    from concourse.bass_utils import run_bass_kernel_spmd

    nc = get_nc()
    in_maps = make_in_maps(x, W_att, W_proj)
    res = run_bass_kernel_spmd(nc, in_maps, list(range(NC_CORES)))
    return combine_outputs(res.results, b_proj)


# revision 10
# speedup vs baseline: 1.0338x; 1.0338x over previous
"""Multi-head causal attention (B=4, T=2048, H=16, D=64) on 8 trn2 NeuronCores.

Sharding: core c = (batch b = c//2, head-group hg = c%2 of 8 heads).
Each core computes its batch's QKV projection for its 8 heads, causal
attention, and a partial output projection (contraction over its 512
channels of W_proj). Host sums the two partials per batch and adds bias.

Per-core kernel layout choices:
  - All HBM tensors are host-side re-laid-out so every DMA moves one
    contiguous 2-8KB chunk per partition (descriptor-efficient).
  - K^T, Q^T stored [hd, t] with head-dim on partitions (64 per head, 2
    heads per 128-partition tile); the per-head score matmul pair (K=64)
    sits on disjoint 64-row PE tiles and dual-issues on HW.
  - V stored [t, h*65+d] with a ones column appended per head -> the AV
    matmul O_T = V_aug^T(stationary) x P_T produces softmax denominators
    in row 64 for free.
  - Scores are computed transposed S_T[k, q]; P_T = exp(S_T) feeds AV
    directly; no max subtraction needed (|scores/8| < ~3).
  - Causal masking of diagonal blocks is an in-place affine_select on
    GpSimd (otherwise idle); softmax reciprocal reads PSUM directly.
  - O_T [hd, t] is exactly the lhsT the output projection needs.

Scheduling: the attention inner loop is ACT(exp)-bound while QKV/proj
are PE-bound.  QKV(tb+1)/proj(qb) matmuls are queued as "filler" units
and injected between attention steps via a virtual-clock deficit so the
PE never idles waiting for exps.  Chains are flushed just-in-time one
pair ahead of their consumer.  proj(2) and the ct={0,1} halves of
proj(3) fill the ACT-bound qb=3 pairs; the tail then only runs the
ct={2,3} halves (8 short chains) + adds, keeping the PE warm to the end.
"""

import os
import sys

import numpy as np

F16_NP = np.dtype(np.float16)

if "/opt/trn_rl_repo" not in sys.path:
    sys.path.insert(0, "/opt/trn_rl_repo")

from collections import deque
from contextlib import ExitStack

import concourse.bass as bass
import concourse.bacc as bacc
import concourse.mybir as mybir
import concourse.tile as tile
from concourse._compat import with_exitstack

P = 128
T = 2048
C = 1024
H_PER_CORE = 8
D = 64
DP = D + 1  # V augmented with a ones column
NC_CORES = 8

TB = 4  # t-blocks of 512
QB = 4  # q-blocks of 512
CI = 8  # contraction tiles of 128 over C for QKV proj

F32 = mybir.dt.float32
F16 = mybir.dt.float16

# virtual-clock cost constants (ns) for filler pacing
PE_CYC = 0.4167
ACT_COL = 0.8333
ACT_OVH = 350.0
STEP_PE_OVH = 35.0
MM_NS = 512 * PE_CYC + 20.0


@with_exitstack
def build_attention_kernel(ctx: ExitStack, tc: tile.TileContext):
    nc = tc.nc

    # HBM layouts (host pre-permuted for contiguous per-partition lines):
    #   x4[ci, tb, co, t]   : x^T channel c = co*128+ci, time t = tb*512+ts
    #   wk4/wq4[mc, ci, co, m]: W[:, mc*128+m] column-chunked
    #   wv2[ci, co, m]      : full [1024, 512] V weight
    #   wp2[ci, co, n]      : [512, 1024] proj weight
    #   y4[p, tt, n]        : output, t = tt*128 + p
    x4 = nc.declare_dram_parameter("x4", [P, TB, CI, 512], F16, isOutput=False)
    wk4 = nc.declare_dram_parameter("wk4", [4, P, CI, P], F16, isOutput=False)
    wq4 = nc.declare_dram_parameter("wq4", [4, P, CI, P], F16, isOutput=False)
    wv2 = nc.declare_dram_parameter("wv2", [P, CI, 512], F16, isOutput=False)
    wp2 = nc.declare_dram_parameter("wp2", [P, 4, C], F16, isOutput=False)
    y4 = nc.declare_dram_parameter("y4", [P, 16, C], F16, isOutput=True)

    # ---- pools ----
    const_pool = ctx.enter_context(tc.tile_pool(name="constp", bufs=1))
    kt_pool = ctx.enter_context(tc.tile_pool(name="ktp", bufs=16))
    qt_pool = ctx.enter_context(tc.tile_pool(name="qtp", bufs=16))
    ot_pool = ctx.enter_context(tc.tile_pool(name="otp", bufs=16))
    v_pool = ctx.enter_context(tc.tile_pool(name="vp", bufs=4))
    w_pool = ctx.enter_context(tc.tile_pool(name="wp_", bufs=1))
    xt_pool = ctx.enter_context(tc.tile_pool(name="xtp", bufs=4))
    pt_pool = ctx.enter_context(tc.tile_pool(name="ptp", bufs=8))
    recip_pool = ctx.enter_context(tc.tile_pool(name="recipp", bufs=6))
    bc_pool = ctx.enter_context(tc.tile_pool(name="bcp", bufs=4))
    y_pool = ctx.enter_context(tc.tile_pool(name="yp", bufs=4))
    yh_pool = ctx.enter_context(tc.tile_pool(name="yhp", bufs=8))
    # PSUM: 8 banks total: s-pairs 2x2, AV accum 2x1, qkv/proj 2x1
    ps_s_pool = ctx.enter_context(tc.tile_pool(name="ps_s", bufs=2, space="PSUM"))
    ps_o_pool = ctx.enter_context(tc.tile_pool(name="ps_o", bufs=2, space="PSUM"))
    ps_q_pool = ctx.enter_context(tc.tile_pool(name="ps_q", bufs=2, space="PSUM"))

    # KT[pt][tb], QT[pt][qb]: [128, 512]; partitions = 2 heads x 64 dims
    KT = [[kt_pool.tile([P, 512], F16, tag="kt", name=f"KT_{pt}_{tb}") for tb in range(TB)] for pt in range(4)]
    QT = [[qt_pool.tile([P, 512], F16, tag="qt", name=f"QT_{pt}_{qb}") for qb in range(QB)] for pt in range(4)]
    OT = [[ot_pool.tile([P, 512], F16, tag="ot", name=f"OT_{hp}_{qb}") for qb in range(QB)] for hp in range(4)]
    V = [v_pool.tile([P, 4, H_PER_CORE * DP], F16, tag="v", name=f"V_{tb}") for tb in range(TB)]
    masks = const_pool.tile([P, 4, 512], F16, tag="masks", name="masks")
    wk_sb = w_pool.tile([P, 4, CI, P], F16)
    wq_sb = w_pool.tile([P, 4, CI, P], F16)
    wv_sb = w_pool.tile([P, CI, 512], F16)
    wp_sb = w_pool.tile([P, 4, C], F16)

    # ---- prologue DMAs: three HWDGE queues, need-ordered; every transfer
    # is per-partition contiguous so descriptors are 2-8KB each ----
    xts = {}
    for tb in range(TB):
        xts[tb] = xt_pool.tile([P, CI, 512], F16, tag="xt", name=f"xt{tb}")
    # sync + scalar (HWDGE) queues carry the critical path, need-ordered;
    # gpsimd (SWDGE) carries wv/wp which have several-us of slack
    nc.sync.dma_start(wk_sb[:, 0], wk4[0])
    nc.scalar.dma_start(xts[0][:, 0:2], x4[:, 0, 0:2])
    nc.sync.dma_start(xts[0][:, 4:6], x4[:, 0, 4:6])
    nc.scalar.dma_start(xts[0][:, 2:4], x4[:, 0, 2:4])
    nc.sync.dma_start(xts[0][:, 6:8], x4[:, 0, 6:8])
    nc.scalar.dma_start(wq_sb[:, 0], wq4[0])
    nc.gpsimd.dma_start(wv_sb[:, :, 0:256], wv2[:, :, 0:256])
    nc.gpsimd.dma_start(wv_sb[:, :, 256:512], wv2[:, :, 256:512])
    nc.sync.dma_start(wk_sb[:, 1], wk4[1])
    nc.scalar.dma_start(wq_sb[:, 1], wq4[1])
    nc.sync.dma_start(xts[1][:, 4:8], x4[:, 1, 4:8])
    nc.scalar.dma_start(xts[1][:, 0:4], x4[:, 1, 0:4])
    nc.sync.dma_start(wk_sb[:, 2:4], wk4[2:4].rearrange("mc ci co m -> ci mc co m"))
    nc.scalar.dma_start(wq_sb[:, 2:4], wq4[2:4].rearrange("mc ci co m -> ci mc co m"))
    nc.gpsimd.dma_start(wp_sb[:], wp2[:])
    nc.sync.dma_start(xts[2][:, 4:8], x4[:, 2, 4:8])
    nc.scalar.dma_start(xts[2][:, 0:4], x4[:, 2, 0:4])
    nc.sync.dma_start(xts[3][:, 4:8], x4[:, 3, 4:8])
    nc.scalar.dma_start(xts[3][:, 0:4], x4[:, 3, 0:4])

    # diagonal causal masks: masks[:, j, :][kk, qq] = 1.0 if qq >= kk + j*128
    for j in range(4):
        nc.gpsimd.memset(masks[:, j, :], 1.0)
        nc.gpsimd.affine_select(
            out=masks[:, j, :],
            in_=masks[:, j, :],
            compare_op=mybir.AluOpType.is_ge,
            fill=0.0,
            base=-j * P,
            pattern=[[1, 512]],
            channel_multiplier=-1,
        )
    # ones column of V
    for tb in range(TB):
        ones_col = V[tb].rearrange("p s (h e) -> p s h e", e=DP)[:, :, :, D : D + 1]
        nc.gpsimd.memset(ones_col, 1.0)

    # ================= filler machinery =================
    filler_q = deque()  # entries: (tag, sub, pe_ns, closure)
    clk = {"deficit": 0.0}

    def emit_fillers(gap_ns):
        clk["deficit"] += gap_ns
        while clk["deficit"] > 0.0 and filler_q:
            _, _, pe_ns, fn = filler_q.popleft()
            fn()
            clk["deficit"] -= pe_ns

    def flush_sub(tag, subs):
        """Emit from the front until no unit with (tag, sub in subs) remains."""
        while any(e[0] == tag and e[1] in subs for e in filler_q):
            _, _, _, fn = filler_q.popleft()
            fn()

    def drain_all():
        while filler_q:
            _, _, _, fn = filler_q.popleft()
            fn()

    # ================= QKV chains =================
    def qkv_chain_units(tb, kind, idx, sub):
        st = {}
        tag = f"qkv{tb}"
        units = []

        def mk(ci):
            def f():
                xt = xts[tb]
                if ci == 0:
                    st["ps"] = ps_q_pool.tile([P, 512], F32, tag="psq", name=f"ps_{tag}")
                ps = st["ps"]
                if kind == "K":
                    nc.tensor.matmul(
                        ps[:], lhsT=wk_sb[:, idx, ci],
                        rhs=xt[:, ci, :], start=(ci == 0), stop=(ci == CI - 1),
                    )
                elif kind == "Q":
                    nc.tensor.matmul(
                        ps[:], lhsT=wq_sb[:, idx, ci],
                        rhs=xt[:, ci, :], start=(ci == 0), stop=(ci == CI - 1),
                    )
                else:  # V
                    nc.tensor.matmul(
                        ps[:], lhsT=xt[:, ci, idx * P : (idx + 1) * P],
                        rhs=wv_sb[:, ci, :], start=(ci == 0), stop=(ci == CI - 1),
                    )
                if ci == CI - 1:
                    if kind == "K":
                        nc.vector.tensor_copy(KT[idx][tb][:], ps[:])
                    elif kind == "Q":
                        nc.vector.tensor_copy(QT[idx][tb][:], ps[:])
                    else:
                        nc.vector.tensor_copy(
                            V[tb][:, idx].rearrange("p (h e) -> p h e", e=DP)[:, :, :D],
                            ps.rearrange("p (h d) -> p h d", d=D),
                        )
            return f

        for ci in range(CI):
            units.append((tag, sub, MM_NS, mk(ci)))
        return units

    def v_half_chain_units(tb, idx, h0):
        """V chain for heads [2*h0, 2*h0+4): N=256 half-width (startup only)."""
        st = {}

        def mk(ci):
            def f():
                xt = xts[tb]
                if ci == 0:
                    st["ps"] = ps_q_pool.tile([P, 512], F32, tag="psq", name="ps_vh")
                ps = st["ps"]
                nc.tensor.matmul(
                    ps[:, :256], lhsT=xt[:, ci, idx * P : (idx + 1) * P],
                    rhs=wv_sb[:, ci, h0 * 256 : (h0 + 1) * 256],
                    start=(ci == 0), stop=(ci == CI - 1),
                )
                if ci == CI - 1:
                    nc.vector.tensor_copy(
                        V[tb][:, idx].rearrange("p (h e) -> p h e", e=DP)[
                            :, 4 * h0 : 4 * h0 + 4, :D
                        ],
                        ps[:, :256].rearrange("p (h d) -> p h d", d=D),
                    )
            return f

        return [(f"qkv{tb}", "v", 256 * PE_CYC + 20.0, mk(ci)) for ci in range(CI)]

    def qkv_units(tb):
        """V first (long-lead deps), then per-pair (Q,K) chains."""
        units = []
        for ts in range(4):
            units += qkv_chain_units(tb, "V", ts, "v")
        for pt in range(4):
            units += qkv_chain_units(tb, "Q", pt, f"p{pt}")
            units += qkv_chain_units(tb, "K", pt, f"p{pt}")
        return units

    # ================= attention =================
    def attention_pair(qb, hp, mid_emit=None):
        # JIT prefetch: flush this pair's chains (safety) plus the next
        # pair's, so the PSUM->SBUF casts land one pair ahead of use.
        subs = {f"p{hp}"}
        if hp == 0:
            subs |= {"v", "p1"}
        elif hp < 3:
            subs.add(f"p{hp + 1}")
        flush_sub(f"qkv{qb}", subs)

        ot_ps = [
            ps_o_pool.tile([DP, 512], F32, tag="ot_ps", name=f"ot_ps_{qb}_{hp}_{i}")
            for i in range(2)
        ]
        nkt = 4 * (qb + 1)
        pts = {}

        def emit_scores_exp(kt):
            tb = kt // 4
            qs = (kt - 4 * qb) * P if kt >= 4 * qb else 0
            nq = 512 - qs
            s_ps = ps_s_pool.tile([P, 2, 512], F32, tag="s_ps", name="s_ps")
            for h2 in range(2):
                nc.tensor.matmul(
                    s_ps[:, h2, qs:],
                    lhsT=KT[hp][tb][
                        h2 * D : (h2 + 1) * D,
                        (kt % 4) * P : (kt % 4 + 1) * P,
                    ],
                    rhs=QT[hp][qb][h2 * D : (h2 + 1) * D, qs:],
                    start=True,
                    stop=True,
                )
            p_t = pt_pool.tile([P, 2, 512], F16, tag="pt", name="p_t")
            nc.scalar.activation(
                p_t[:, :, qs:],
                s_ps[:, :, qs:],
                mybir.ActivationFunctionType.Exp,
                scale=0.125,
            )
            if kt >= 4 * qb:  # diagonal: zero q < k entries
                j = kt - 4 * qb
                mb = masks[:, j : j + 1, qs:].to_broadcast([P, 2, nq])
                nc.vector.tensor_mul(p_t[:, :, qs:], p_t[:, :, qs:], mb)
            pts[kt] = (p_t, qs, nq)
            return nq

        def emit_av(kt):
            tb = kt // 4
            p_t, qs, nq = pts.pop(kt)
            for h2 in range(2):
                h = 2 * hp + h2
                nc.tensor.matmul(
                    ot_ps[h2][:, qs:],
                    lhsT=V[tb][:, kt % 4, h * DP : (h + 1) * DP],
                    rhs=p_t[:, h2, qs:],
                    start=(kt == 0),
                    stop=(kt == nkt - 1),
                )
            return nq

        if mid_emit is not None:
            # split form (startup): all scores/exps first so ACT streams
            # while V is still loading, then the AVs
            for kt in range(nkt):
                nq = emit_scores_exp(kt)
                emit_fillers(2 * nq * ACT_COL + ACT_OVH - nq * PE_CYC - STEP_PE_OVH)
            mid_emit()
            for kt in range(nkt):
                emit_av(kt)
        else:
            # software pipeline: S(kt+1) before AV(kt); fillers pace the gaps
            nq = emit_scores_exp(0)
            emit_fillers(2 * nq * ACT_COL + ACT_OVH - nq * PE_CYC - STEP_PE_OVH)
            for kt in range(1, nkt):
                nq_s = emit_scores_exp(kt)
                nq_a = emit_av(kt - 1)
                emit_fillers(
                    2 * nq_s * ACT_COL + ACT_OVH
                    - (nq_s + 2 * nq_a) * PE_CYC - STEP_PE_OVH
                )
            emit_av(nkt - 1)

        # normalize: divide rows 0..63 by the sums row (64)
        for h2 in range(2):
            recip = recip_pool.tile([1, 512], F32, tag="recip", name="recip")
            nc.vector.tensor_copy(recip[:], ot_ps[h2][D : D + 1, :])
            nc.vector.reciprocal_approx_fast(recip[:], recip[:])
            bc = bc_pool.tile([D, 512], F32, tag="bc", name="bc")
            nc.gpsimd.partition_broadcast(bc[:], recip[:])
            nc.vector.tensor_mul(
                OT[hp][qb][h2 * D : (h2 + 1) * D, :],
                ot_ps[h2][:D, :],
                bc[:],
            )

    # ================= output projection =================
    # qb<3: full 4-ct chains -> y tile -> per-tt DMA.
    # qb=3: ct{0,1} half-chains run as fillers during the ACT-bound final
    # pairs (partial saved to SBUF); ct{2,3} half-chains + add at the tail.
    ysbs = {}
    yhalf = {}

    def proj_units(qb):
        tag = f"proj{qb}"
        units = []
        proj_ps = {}

        def mk(tt, nb, ct):
            def f():
                if nb == 0 and ct == 0:
                    ysbs[tt] = y_pool.tile([P, C], F16, tag="ysb", name="ysb")
                st_key = (tt, nb)
                if ct == 0:
                    proj_ps[st_key] = ps_q_pool.tile(
                        [P, 512], F32, tag="psq", name=f"ps_{tag}"
                    )
                ps = proj_ps[st_key]
                nc.tensor.matmul(
                    ps[:],
                    lhsT=OT[ct][qb][:, (tt % 4) * P : (tt % 4 + 1) * P],
                    rhs=wp_sb[:, ct, nb * 512 : (nb + 1) * 512],
                    start=(ct == 0),
                    stop=(ct == 3),
                )
                if ct == 3:
                    proj_ps.pop(st_key)
                    nc.vector.tensor_copy(
                        ysbs[tt][:, nb * 512 : (nb + 1) * 512], ps[:]
                    )
                    if nb == 1:
                        nc.sync.dma_start(y4[:, tt], ysbs[tt][:])
            return f

        for tt in range(4 * qb, 4 * qb + 4):
            for nb in range(2):
                for ct in range(4):
                    units.append((tag, "p", MM_NS, mk(tt, nb, ct)))
        return units

    def proj3_first_half_units():
        """ct=0,1 partial chains for qb=3; result parked in SBUF."""
        units = []
        proj_ps = {}

        def mk(tt, nb, ct):
            def f():
                st_key = (tt, nb)
                if ct == 0:
                    proj_ps[st_key] = ps_q_pool.tile(
                        [P, 512], F32, tag="psq", name="ps_p3a"
                    )
                ps = proj_ps[st_key]
                nc.tensor.matmul(
                    ps[:],
                    lhsT=OT[ct][3][:, (tt % 4) * P : (tt % 4 + 1) * P],
                    rhs=wp_sb[:, ct, nb * 512 : (nb + 1) * 512],
                    start=(ct == 0),
                    stop=(ct == 1),
                )
                if ct == 1:
                    proj_ps.pop(st_key)
                    yh = yh_pool.tile([P, 512], F16, tag="yh", name="yh")
                    nc.vector.tensor_copy(yh[:], ps[:])
                    yhalf[st_key] = yh
            return f

        for tt in range(12, 16):
            for nb in range(2):
                for ct in range(2):
                    units.append(("proj3a", "p", MM_NS, mk(tt, nb, ct)))
        return units

    def emit_proj3_tail():
        """ct=2,3 half-chains + adds + stores, in two groups of 4 chains
        (2 ps_s tiles per group).  Each group's ct=2 MMs go first — their
        OT[2] dep is ready before norm(3,3) — then ct=3 MMs, adds, DMAs."""
        chains = [(tt, nb) for tt in range(12, 16) for nb in range(2)]
        for g in range(2):
            grp = chains[4 * g : 4 * g + 4]
            proj_ps = {}
            for i, (tt, nb) in enumerate(grp):
                if i % 2 == 0:
                    ps2 = ps_s_pool.tile([P, 2, 512], F32, tag="s_ps", name="ps_p3b")
                ps = ps2[:, i % 2, :]
                proj_ps[(tt, nb)] = ps
                nc.tensor.matmul(
                    ps,
                    lhsT=OT[2][3][:, (tt % 4) * P : (tt % 4 + 1) * P],
                    rhs=wp_sb[:, 2, nb * 512 : (nb + 1) * 512],
                    start=True, stop=False,
                )
            for tt, nb in grp:
                if nb == 0:
                    ysbs[tt] = y_pool.tile([P, C], F16, tag="ysb", name="ysb")
                ps = proj_ps[(tt, nb)]
                nc.tensor.matmul(
                    ps,
                    lhsT=OT[3][3][:, (tt % 4) * P : (tt % 4 + 1) * P],
                    rhs=wp_sb[:, 3, nb * 512 : (nb + 1) * 512],
                    start=False, stop=True,
                )
                nc.vector.tensor_add(
                    ysbs[tt][:, nb * 512 : (nb + 1) * 512], ps, yhalf.pop((tt, nb))[:]
                )
                if nb == 1:
                    nc.sync.dma_start(y4[:, tt], ysbs[tt][:])

    # ================= master schedule =================
    # tb=0: K0,V0,Q0 then pair(0,0) ASAP; V1-3 and later K/Q chains slot
    # between the early (cheap) pairs, one pair ahead of their consumers.
    def emit_chains(units):
        for u in units:
            u[3]()

    emit_chains(qkv_chain_units(0, "K", 0, "p0"))
    emit_chains(qkv_chain_units(0, "Q", 0, "p0"))
    for ts in range(4):
        emit_chains(v_half_chain_units(0, ts, 0))
    filler_q.extend(qkv_units(1))
    for hp in range(4):
        if hp == 1:  # second V half feeds pairs hp>=2, one pair ahead
            for ts in range(4):
                emit_chains(v_half_chain_units(0, ts, 1))
        if hp < 3:
            emit_chains(qkv_chain_units(0, "K", hp + 1, f"p{hp+1}"))
            emit_chains(qkv_chain_units(0, "Q", hp + 1, f"p{hp+1}"))
        attention_pair(0, hp)

    for qb in range(1, 4):
        if qb < 3:
            filler_q.extend(qkv_units(qb + 1))
            filler_q.extend(proj_units(qb - 1))
        else:
            # qb=3 is ACT-bound with no QKV left: fill it with proj(2)
            # and the first halves of proj(3)
            filler_q.extend(proj_units(2))
            filler_q.extend(proj3_first_half_units())
        for hp in range(4):
            attention_pair(qb, hp)
    drain_all()
    emit_proj3_tail()

    return nc


_CACHED_NC = None


def get_nc():
    global _CACHED_NC
    if _CACHED_NC is None:
        nc = bacc.Bacc()
        with tile.TileContext(nc) as tc:
            build_attention_kernel(tc)
        nc.compile()
        _CACHED_NC = nc
    return _CACHED_NC


def make_in_maps(x, W_att, W_proj):
    x = np.asarray(x, dtype=np.float32)
    W_att = np.asarray(W_att, dtype=np.float32)
    W_proj = np.asarray(W_proj, dtype=np.float32)
    in_maps = []
    for c in range(NC_CORES):
        b, hg = c // 2, c % 2
        s = hg * 512
        # x4[ci, tb, co, t]: x^T[c, t] with c = co*128+ci, t = tb*512+ts
        xT = np.ascontiguousarray(x[b].T).astype(F16_NP)  # [1024, 2048]
        x4 = np.ascontiguousarray(
            xT.reshape(CI, P, TB, 512).transpose(1, 2, 0, 3)
        )
        # w4[mc, ci, co, m]: W[c, s + mc*128 + m]
        def w4(w):  # w: [1024, 512]
            return np.ascontiguousarray(
                w.reshape(CI, P, 4, P).transpose(2, 1, 0, 3).astype(F16_NP)
            )
        wk = W_att[:, 0 * C + s : 0 * C + s + 512].astype(F16_NP)
        wq = W_att[:, 1 * C + s : 1 * C + s + 512].astype(F16_NP)
        wv = W_att[:, 2 * C + s : 2 * C + s + 512].astype(F16_NP)
        # wv2[ci, co, m]
        wv2 = np.ascontiguousarray(wv.reshape(CI, P, 512).transpose(1, 0, 2))
        # wp2[ci, co, n]: W_proj[s + co*128 + ci, n]
        wp2 = np.ascontiguousarray(
            W_proj[s : s + 512].astype(F16_NP).reshape(4, P, C).transpose(1, 0, 2)
        )
        in_maps.append(
            {"x4": x4, "wk4": w4(wk), "wq4": w4(wq), "wv2": wv2, "wp2": wp2}
        )
    return in_maps


def combine_outputs(results, b_proj):
    B = NC_CORES // 2
    out = np.empty((B, T, C), dtype=np.float32)
    bias = np.asarray(b_proj, dtype=np.float32)
    for b in range(B):
        # y4[p, tt, n] -> y[t, n] with t = tt*128 + p
        ya = results[2 * b]["y4"].astype(np.float32).transpose(1, 0, 2).reshape(T, C)
        yb = results[2 * b + 1]["y4"].astype(np.float32).transpose(1, 0, 2).reshape(T, C)
        out[b] = ya + yb + bias
    return out


def kernel(x, W_att, W_proj, b_proj):
    from concourse.bass_utils import run_bass_kernel_spmd

    nc = get_nc()
    in_maps = make_in_maps(x, W_att, W_proj)
    res = run_bass_kernel_spmd(nc, in_maps, list(range(NC_CORES)))
    return combine_outputs(res.results, b_proj)
